# revision 1
# baseline (speedup 1.0000x reference)
"""MaxRecallLoss Trainium2 kernel: 8-core data-parallel Bass/Tile implementation.

Full inputs in, full (scalar) output out. Internally: shard logits/targets
across 8 NeuronCores along batch; each core computes per-partition partial
sums (per-class weighted-CE buckets, soft-TP sum, cancer count); host
combines the 8x[128,64] partials into the final scalar.

Layout strategy: logits tiles [128, RT, 8] (f32, contiguous DMA) are
deinterleaved by the Scalar engine into per-class-contiguous bf16 tensors
[128, 8, RT] (plain copy, exp(x/1.5), exp(x)), so all per-row Vector-engine
work runs on contiguous bf16 slices at 2x/4x DVE modes. Per-class bucket
accumulation (accum_out) folds base_weight[t] gathering into host algebra.
"""
import os
import sys

try:
    import concourse.bass as bass  # noqa: F401
except ImportError:
    sys.path.insert(0, "/opt/trn_rl_repo")

import numpy as np

import concourse.bass as bass
import concourse.tile as tile
from concourse import bacc, mybir
from concourse.bass_utils import run_bass_kernel_spmd

F32 = mybir.dt.float32
BF16 = mybir.dt.bfloat16
I32 = mybir.dt.int32
ALU = mybir.AluOpType
ACTF = mybir.ActivationFunctionType
AXL = mybir.AxisListType

B = 2097152
C = 8
NCORES = 8
RPC = B // NCORES          # rows per core = 262144
P = 128
RPP = RPC // P             # rows per partition = 2048
NTILES = 4
RT = RPP // NTILES         # rows per partition per tile = 512

TEMP = 1.5
CSM, BSM = 0.05, 0.1
RECALL_W = 0.5

# ce = lse - (cs(t)*S + ct(t)*x_t + cc(t)*Sc), coefficients by target class
# (the 1/TEMP is folded in). cancer rows (t in {0,1,3}): smooth=.05, norm=1
CS_C = (CSM / C) / TEMP
CT_C = (1.0 - CSM) / TEMP
CC_C = 0.0
# benign rows: smooth=.1, extra=.1/6, norm=1.05
_norm_b = 1.0 + 3.0 * (BSM * 0.5 / 3.0)
CS_B = (BSM / C) / _norm_b / TEMP
CT_B = (1.0 - BSM) / _norm_b / TEMP
CC_B = (BSM * 0.5 / 3.0) / _norm_b / TEMP
CS_D = CS_C - CS_B
CT_D = CT_C - CT_B
CC_D = CC_C - CC_B

REPEAT = int(os.environ.get("KREPEAT", "1"))

_NC = None


def _body(nc, tc, xin, tin, out):
    import contextlib
    ctx = contextlib.ExitStack()
    with ctx:
        singles = ctx.enter_context(tc.tile_pool(name="singles", bufs=1))
        xpool = ctx.enter_context(tc.tile_pool(name="xpool", bufs=2))
        dpool = ctx.enter_context(tc.tile_pool(name="dpool", bufs=2))
        tmp = ctx.enter_context(tc.tile_pool(name="tmp", bufs=1))
        junkp = ctx.enter_context(tc.tile_pool(name="junkp", bufs=2))

        xg = xin.rearrange("(p r) c -> p r c", p=P)      # [128, RPP, 8]
        tg = tin.rearrange("(p r) -> p r", p=P)          # [128, RPP]

        tgt_i = singles.tile([P, RPP], I32)
        nc.sync.dma_start(tgt_i[:], tg[:, :])
        tfb = singles.tile([P, RPP], BF16)
        nc.vector.tensor_copy(tfb[:], tgt_i[:])

        stats = singles.tile([P, 64], F32)
        nc.vector.memset(stats[:], 0.0)

        def _tiles():
            for k in range(NTILES):
                X = xpool.tile([P, RT, C], F32, tag="x", name="x")
                nc.sync.dma_start(X[:], xg[:, k * RT:(k + 1) * RT, :])
                tfk = tfb[:, k * RT:(k + 1) * RT]

                def T(name, ch=1, dt=BF16, lead=False):
                    if ch == 1:
                        return tmp.tile([P, RT], dt, tag=name, name=name)
                    if lead:
                        return tmp.tile([P, ch, RT], dt, tag=name, name=name)
                    return tmp.tile([P, RT, ch], dt, tag=name, name=name)

                # ScalarE passes, contiguous bf16 outputs [128, RT, 8]
                Xf = X.rearrange("p r c -> p (r c)")
                Xb = dpool.tile([P, RT, C], BF16, tag="xb", name="xb")
                nc.scalar.activation(Xb.rearrange("p r c -> p (r c)"), Xf,
                                     ACTF.Copy)
                u8 = dpool.tile([P, RT, C], BF16, tag="u8", name="u8")
                nc.scalar.activation(u8.rearrange("p r c -> p (r c)"), Xf,
                                     ACTF.Exp, scale=1.0 / TEMP)
                v8 = dpool.tile([P, RT, C], BF16, tag="v8", name="v8")
                nc.scalar.activation(v8.rearrange("p r c -> p (r c)"), Xf,
                                     ACTF.Exp)

                # pairwise-tree sums on contiguous 4-run slices (bf16 2x)
                eL1 = T("eL1", 4)
                nc.vector.tensor_add(eL1[:], u8[:, :, 0:4], u8[:, :, 4:8])
                eL2 = T("eL2", 2)
                nc.vector.tensor_add(eL2[:], eL1[:, :, 0:2], eL1[:, :, 2:4])
                E15 = T("E15")
                nc.vector.tensor_add(E15[:], eL2[:, :, 0], eL2[:, :, 1])
                lse = T("lse")
                nc.scalar.activation(lse[:], E15[:], ACTF.Ln)

                fL1 = T("fL1", 4)
                nc.vector.tensor_add(fL1[:], v8[:, :, 0:4], v8[:, :, 4:8])
                fL2 = T("fL2", 2)
                nc.vector.tensor_add(fL2[:], fL1[:, :, 0:2], fL1[:, :, 2:4])
                E1 = T("E1")
                nc.vector.tensor_add(E1[:], fL2[:, :, 0], fL2[:, :, 1])
                Ec1 = T("Ec1")
                nc.vector.tensor_add(Ec1[:], v8[:, :, 0], v8[:, :, 1])
                nc.vector.tensor_add(Ec1[:], Ec1[:], v8[:, :, 3])

                sL1 = T("sL1", 4)
                nc.vector.tensor_add(sL1[:], Xb[:, :, 0:4], Xb[:, :, 4:8])
                sL2 = T("sL2", 2)
                nc.vector.tensor_add(sL2[:], sL1[:, :, 0:2], sL1[:, :, 2:4])
                S = T("S")
                nc.vector.tensor_add(S[:], sL2[:, :, 0], sL2[:, :, 1])
                Sc = T("Sc")
                nc.vector.tensor_add(Sc[:], Xb[:, :, 0], Xb[:, :, 1])
                nc.vector.tensor_add(Sc[:], Sc[:], Xb[:, :, 3])

                # maxes (strided slices, 1x)
                Mc = T("Mc")
                nc.vector.tensor_tensor(Mc[:], Xb[:, :, 0], Xb[:, :, 1], op=ALU.max)
                nc.vector.tensor_tensor(Mc[:], Mc[:], Xb[:, :, 3], op=ALU.max)
                n1 = T("n1")
                nc.vector.tensor_tensor(n1[:], Xb[:, :, 4], Xb[:, :, 5], op=ALU.max)
                n2 = T("n2")
                nc.vector.tensor_tensor(n2[:], Xb[:, :, 6], Xb[:, :, 7], op=ALU.max)
                Mnc = T("Mnc")
                nc.vector.tensor_tensor(Mnc[:], n1[:], n2[:], op=ALU.max)
                nc.vector.tensor_tensor(Mnc[:], Mnc[:], Xb[:, :, 2], op=ALU.max)
                M8 = T("M8")
                nc.vector.tensor_tensor(M8[:], Mc[:], Mnc[:], op=ALU.max)

                # x_t gather: per-class terms written contiguous [128, 8, RT]
                xterm = T("xterm", 8, lead=True)
                for c in range(C):
                    nc.vector.scalar_tensor_tensor(
                        out=xterm[:, c, :], in0=tfk, scalar=float(c),
                        in1=Xb[:, :, c], op0=ALU.is_equal, op1=ALU.mult)
                xL1 = T("xL1", 4, lead=True)
                nc.vector.tensor_add(xL1[:], xterm[:, 0:4, :], xterm[:, 4:8, :])
                xL2 = T("xL2", 2, lead=True)
                nc.vector.tensor_add(xL2[:], xL1[:, 0:2, :], xL1[:, 2:4, :])
                xt = T("xt")
                nc.vector.tensor_add(xt[:], xL2[:, 0, :], xL2[:, 1, :])

                # flags
                icp = T("icp")
                nc.vector.tensor_tensor(icp[:], Mc[:], Mnc[:], op=ALU.is_ge)
                eq = T("eq")
                nc.vector.tensor_tensor(eq[:], xt[:], M8[:], op=ALU.is_ge)
                i01 = T("i01")
                nc.vector.tensor_scalar(i01[:], tfk, 1.0, None, op0=ALU.is_le)
                e3 = T("e3")
                nc.vector.tensor_scalar(e3[:], tfk, 3.0, None, op0=ALU.is_equal)
                isc = T("isc")
                nc.vector.tensor_add(isc[:], i01[:], e3[:])
                is0 = T("is0")
                nc.vector.tensor_scalar(is0[:], tfk, 0.5, None, op0=ALU.is_le)

                # m1 = isc*(5 + 4*is0 - icp*(4 + 4*is0 + eq));  g = 1 + m1
                aa = T("aa")
                nc.vector.tensor_scalar(aa[:], is0[:], 4.0, 4.0,
                                        op0=ALU.mult, op1=ALU.add)
                a2 = T("a2")
                nc.vector.tensor_add(a2[:], aa[:], eq[:])
                a3 = T("a3")
                nc.vector.tensor_mul(a3[:], icp[:], a2[:])
                a4 = T("a4")
                nc.vector.tensor_scalar(a4[:], aa[:], 1.0, None, op0=ALU.add)
                a5 = T("a5")
                nc.vector.tensor_tensor(a5[:], a4[:], a3[:], op=ALU.subtract)
                m1 = T("m1")
                nc.vector.tensor_mul(m1[:], isc[:], a5[:])

                # P = lse - (CS_B S + CC_B Sc + CT_B xt) - isc*(CS_D S + CC_D Sc + CT_D xt)
                q1 = T("q1")
                nc.vector.tensor_scalar(q1[:], S[:], CS_B, None, op0=ALU.mult)
                q2 = T("q2")
                nc.vector.scalar_tensor_tensor(q2[:], in0=Sc[:], scalar=CC_B,
                                               in1=q1[:], op0=ALU.mult, op1=ALU.add)
                q3 = T("q3")
                nc.vector.scalar_tensor_tensor(q3[:], in0=xt[:], scalar=CT_B,
                                               in1=q2[:], op0=ALU.mult, op1=ALU.add)
                P0 = T("P0")
                nc.vector.tensor_tensor(P0[:], lse[:], q3[:], op=ALU.subtract)
                qd1 = T("qd1")
                nc.vector.tensor_scalar(qd1[:], S[:], CS_D, None, op0=ALU.mult)
                qd2 = T("qd2")
                nc.vector.scalar_tensor_tensor(qd2[:], in0=Sc[:], scalar=CC_D,
                                               in1=qd1[:], op0=ALU.mult, op1=ALU.add)
                qd3 = T("qd3")
                nc.vector.scalar_tensor_tensor(qd3[:], in0=xt[:], scalar=CT_D,
                                               in1=qd2[:], op0=ALU.mult, op1=ALU.add)
                md = T("md")
                nc.vector.tensor_mul(md[:], isc[:], qd3[:])
                Pr = T("Pr")
                nc.vector.tensor_tensor(Pr[:], P0[:], md[:], op=ALU.subtract)

                # gP = (m1 + 1) * P
                gP = T("gP")
                nc.vector.scalar_tensor_tensor(gP[:], in0=m1[:], scalar=1.0,
                                               in1=Pr[:], op0=ALU.add, op1=ALU.mult)

                # per-class buckets of gP (base_weight applied on host)
                for c in range(C):
                    jb = junkp.tile([P, RT], BF16, tag=f"jb{c % 2}", name="jb")
                    nc.vector.scalar_tensor_tensor(
                        out=jb[:], in0=tfk, scalar=float(c), in1=gP[:],
                        op0=ALU.is_equal, op1=ALU.mult,
                        accum_out=stats[:, 8 * k + c:8 * k + c + 1])

                # soft-tp and count
                rE1 = T("rE1", dt=F32)
                nc.vector.reciprocal(rE1[:], E1[:])
                pcm = T("pcm")
                nc.vector.tensor_mul(pcm[:], Ec1[:], rE1[:])
                jt = junkp.tile([P, RT], BF16, tag="jt", name="jt")
                nc.vector.scalar_tensor_tensor(
                    out=jt[:], in0=isc[:], scalar=1.0, in1=pcm[:],
                    op0=ALU.mult, op1=ALU.mult,
                    accum_out=stats[:, 32 + k:33 + k])
                jc = junkp.tile([P, RT], BF16, tag="jc", name="jc")
                nc.vector.tensor_scalar(
                    out=jc[:], in0=isc[:], scalar1=1.0, scalar2=None,
                    op0=ALU.mult, op1=ALU.add,
                    accum_out=stats[:, 36 + k:37 + k])

        if REPEAT > 1:
            with tc.For_i(0, REPEAT, 1) as _rep:
                _tiles()
        else:
            _tiles()

        nc.sync.dma_start(out[:, :], stats[:])


def _build():
    nc = bacc.Bacc("TRN2", target_bir_lowering=False, debug=False,
                   num_devices=NCORES)
    xin = nc.dram_tensor("logits", [RPC, C], F32, kind="ExternalInput").ap()
    tin = nc.dram_tensor("tgt", [RPC], I32, kind="ExternalInput").ap()
    out = nc.dram_tensor("out", [P, 64], F32, kind="ExternalOutput").ap()
    with tile.TileContext(nc) as tc:
        _body(nc, tc, xin, tin, out)
    nc.compile()
    return nc


def get_nc():
    global _NC
    if _NC is None:
        _NC = _build()
    return _NC


def kernel(logits, targets, class_counts):
    logits = np.ascontiguousarray(np.asarray(logits, dtype=np.float32))
    targets = np.ascontiguousarray(np.asarray(targets, dtype=np.int32))
    cc = np.asarray(class_counts, dtype=np.float32)

    w = 1.0 / np.sqrt(cc.astype(np.float64) + 1.0)
    bw = w / w.sum() * C  # [8] float64

    nc = get_nc()
    in_maps = []
    for i in range(NCORES):
        sl = slice(i * RPC, (i + 1) * RPC)
        in_maps.append({"logits": logits[sl], "tgt": targets[sl]})
    res = run_bass_kernel_spmd(nc, in_maps, core_ids=list(range(NCORES)))

    wce = 0.0
    tp = 0.0
    cnt = 0.0
    for i in range(NCORES):
        st = res.results[i]["out"].astype(np.float64)
        # per-class gP buckets at [:, 8k+c]; multiply by bw[c] on host
        bucket = st[:, 0:32].sum(axis=0).reshape(NTILES, C).sum(axis=0)  # [C]
        wce += (bucket * bw).sum()
        tp += st[:, 32:36].sum()
        cnt += st[:, 36:40].sum()
    base = wce / B
    fn = cnt - tp
    recall = tp / (tp + fn + 1e-8)
    out = base + RECALL_W * (1.0 - recall)
    return np.float32(out)

